# revision 6
# baseline (speedup 1.0000x reference)
"""Trainium2 Bass kernel for the LocalConnectivity diamond-ring stencil.

out[b, x, y] = sum_{1<=|dx|+|dy|<=5} w[|dx|+|dy|-1] * in[b, (x+dx)%512, (y+dy)%512]

Strategy
--------
Data-parallel over batch: 64 samples -> 8 cores x 8 samples. Per sample the
512x512 grid is processed in 5 row-tiles (~103 output rows each). The whole
60-tap stencil runs on the TensorEngine as 11 PSUM-accumulating matmuls, one
per horizontal shift dy in [-5, 5]:

  psum[p, f] += sum_c  WB_dy[c, p] * X[c, f + dy_idx]

where X is the input tile with 5 halo rows on each side (contraction dim =
nrows+10 partitions) and 5 circular halo columns on each side (horizontal
shifts become free-dim AP offsets), and WB_dy is the banded Toeplitz matrix
holding the vertical taps of kernel column dy: WB_dy[c, p] = K(c-p-5, dy).

float32r keeps the PE at 1 cycle/row while multiplying at FP22 (~2e-4 rel
err). Bulk HBM traffic is issued from GpSimd (software DGE - the only DGE
that fans transfers out across all 16 SDMA engines; the sync/scalar HW DGE
queues serialize on one engine at ~18 GB/s). Interior row-tiles are loaded
by ONE overlapping-window DMA per sample, outputs stored by two merged DMAs
per sample. Circular column halos are filled by on-chip VectorE copies.
"""

import numpy as np

import concourse.bass as bass
import concourse.bacc as bacc
import concourse.mybir as mybir
from concourse import tile
from concourse.bass_utils import run_bass_kernel_spmd

B, H, W = 64, 512, 512
NCORES = 8
BPC = B // NCORES  # samples per core
MAXD = 5
HALO = MAXD
DYS = 2 * MAXD + 1  # 11 horizontal shifts
TR = 103  # rows per tile (last tile: 100)
ROW_TILES = [(0, 103), (103, 103), (206, 103), (309, 103), (412, 100)]
XW = W + 2 * HALO  # 522


def _build_band_weights(dw: np.ndarray) -> np.ndarray:
    """[128, 11*128] f32: WB[c, j*128 + p] = K(c-p-5, j-5)."""
    wb = np.zeros((128, DYS, 128), dtype=np.float32)
    p = np.arange(128)
    for j in range(DYS):
        dy = j - MAXD
        for dx in range(-MAXD, MAXD + 1):
            d = abs(dx) + abs(dy)
            if 1 <= d <= MAXD:
                c = p + dx + HALO
                valid = (c >= 0) & (c < 128)
                wb[c[valid], j, p[valid]] = dw[d - 1]
    return np.ascontiguousarray(wb.reshape(128, DYS * 128))


_CACHED_NC = None


def _build_program():
    f32 = mybir.dt.float32
    f32r = mybir.dt.float32r

    nc = bacc.Bacc(None, target_bir_lowering=False)
    x = nc.dram_tensor("x", [BPC, H, W], f32r, kind="ExternalInput")
    wb = nc.dram_tensor("wb", [128, DYS * 128], f32r, kind="ExternalInput")
    y = nc.dram_tensor("y", [BPC, H, W], f32, kind="ExternalOutput")

    with tile.TileContext(nc) as tc:
        with (
            tc.tile_pool(name="wpool", bufs=1) as wpool,
            tc.tile_pool(name="xmpool", bufs=3) as xmpool,
            tc.tile_pool(name="xepool", bufs=4) as xepool,
            tc.tile_pool(name="opool", bufs=3) as opool,
            tc.tile_pool(name="pspool", bufs=8, space=bass.MemorySpace.PSUM) as pspool,
        ):
            wtile = wpool.tile([128, DYS * 128], f32r)
            nc.sync.dma_start(wtile[:], wb[:])

            for b in range(BPC):
                # ---- interior tiles t=1..3: one overlapping-window DMA ----
                xtm = xmpool.tile([128, 3, XW], f32r)
                dst = xtm[0:113, 0:3, HALO : HALO + W]
                src = x[b, TR - HALO : TR - HALO + 113, :]
                src = bass.AP(
                    src.tensor,
                    src.offset,
                    [src.ap[0], [TR * W, 3], src.ap[1]],
                )
                nc.gpsimd.dma_start(dst, src)
                # circular column halos for all 3 blocks in 2 ops
                nc.vector.tensor_copy(
                    xtm[0:113, 0:3, 0:HALO], xtm[0:113, 0:3, W : W + HALO]
                )
                nc.vector.tensor_copy(
                    xtm[0:113, 0:3, HALO + W :], xtm[0:113, 0:3, HALO : 2 * HALO]
                )

                # ---- edge tiles t=0 and t=4 (row-wrapped) ----
                xt0 = xepool.tile([128, XW], f32r, tag="xt0")
                #   rows 507..511 then 0..107
                nc.sync.dma_start(
                    xt0[0:HALO, HALO : HALO + W], x[b, H - HALO : H, :]
                )
                nc.gpsimd.dma_start(
                    xt0[HALO : HALO + 108, HALO : HALO + W], x[b, 0:108, :]
                )
                nc.vector.tensor_copy(xt0[0:113, 0:HALO], xt0[0:113, W : W + HALO])
                nc.vector.tensor_copy(
                    xt0[0:113, HALO + W :], xt0[0:113, HALO : 2 * HALO]
                )

                xt4 = xepool.tile([128, XW], f32r, tag="xt4")
                #   rows 407..511 then 0..4
                nc.gpsimd.dma_start(
                    xt4[0:105, HALO : HALO + W], x[b, 4 * TR - HALO : H, :]
                )
                nc.sync.dma_start(xt4[105:110, HALO : HALO + W], x[b, 0:HALO, :])
                nc.vector.tensor_copy(xt4[0:110, 0:HALO], xt4[0:110, W : W + HALO])
                nc.vector.tensor_copy(
                    xt4[0:110, HALO + W :], xt4[0:110, HALO : 2 * HALO]
                )

                # ---- 11 accumulating matmuls per tile + eviction ----
                otb = opool.tile([128, 5, W], f32)
                for t, (r0, nrows) in enumerate(ROW_TILES):
                    ctr = nrows + 2 * HALO
                    pt = pspool.tile([128, W], f32)
                    for j in range(DYS):
                        if t == 0:
                            rhs = xt0[0:ctr, j : j + W]
                        elif t == 4:
                            rhs = xt4[0:ctr, j : j + W]
                        else:
                            rhs = xtm[0:ctr, t - 1, j : j + W]
                        nc.tensor.matmul(
                            pt[0:nrows, :],
                            wtile[0:ctr, j * 128 : j * 128 + nrows],
                            rhs,
                            start=(j == 0),
                            stop=(j == DYS - 1),
                        )
                    nc.scalar.copy(otb[0:nrows, t, :], pt[0:nrows, :])

                # ---- merged output DMAs ----
                dst = y[b, 0 : 4 * TR, :]
                dst = bass.AP(
                    dst.tensor,
                    dst.offset,
                    [[W, TR], [TR * W, 4], [1, W]],
                )
                nc.gpsimd.dma_start(dst, otb[0:TR, 0:4, :])
                nc.gpsimd.dma_start(y[b, 4 * TR : H, :], otb[0:100, 4, :])
    nc.compile()
    return nc


def _get_program():
    global _CACHED_NC
    if _CACHED_NC is None:
        _CACHED_NC = _build_program()
    return _CACHED_NC


def _run(grid_spikes, distance_weights, trace=False):
    grid_spikes = np.ascontiguousarray(np.asarray(grid_spikes, dtype=np.float32))
    distance_weights = np.asarray(distance_weights, dtype=np.float32)
    assert grid_spikes.shape == (B, H, W), grid_spikes.shape
    wb_np = _build_band_weights(distance_weights)

    nc = _get_program()
    in_maps = [
        {
            "x": np.ascontiguousarray(grid_spikes[i * BPC : (i + 1) * BPC]),
            "wb": wb_np,
        }
        for i in range(NCORES)
    ]
    res = run_bass_kernel_spmd(nc, in_maps, list(range(NCORES)), trace=trace)
    out = np.concatenate([res.results[i]["y"] for i in range(NCORES)], axis=0)
    return out.astype(np.float32, copy=False), res


def kernel(grid_spikes, distance_weights):
    out, _ = _run(grid_spikes, distance_weights, trace=False)
    return out


def kernel_traced(grid_spikes, distance_weights):
    out, res = _run(grid_spikes, distance_weights, trace=True)
    return out, res
